# revision 1
# baseline (speedup 1.0000x reference)
"""CrossViewLoss (SimCLR-style NT-Xent) on 8 trn2 NeuronCores.

Math: with z = row-normalized emb, only the two cross-view blocks of the
[2N,2N] similarity survive the mask, and both are views of the single
[N,N] matrix S = z_i @ z_j.T:
    loss * 2N = sum_m [log(0.5*rowsum_m) - 4*pos_m]
              + sum_c  log(0.5*colsum_c)
where rowsum/colsum are row/col sums of exp(2*S) and pos = diag(S).

Sharding: rows of S across 8 cores (512 anchor rows each).  Each core
receives its own 512-row slices of emb_i/emb_j, normalizes + transposes
its z_j slice, AllGathers z_j^T so every core holds the full [256,4096]
rhs, computes its row block of exp(2*S) with fused row sums (activation
accum_out) and column-sum accumulation (ones-vector matmuls into PSUM),
then one AllReduce combines per-core colsum partials and scalar
partials; every core finishes the loss identically.  Matmuls run in
float32r (full PE rate at N=512, ~1e-7 end-to-end error here).
"""

import numpy as np

N = 4096
D = 256
C = 8
SLICE = N // C          # 512 rows per core
P = 128
MI = SLICE // P         # 4 row tiles per core
KC = D // P             # 2 contraction chunks
NJT = 512               # n-chunk (PSUM bank) size
NJ = N // NJT           # 8 n-chunks
AR_LEN = N + 8          # colsum[4096] + P_c at [4096] + pad to 32B multiple

_CACHE = {}


def _build_nc(local_sim=False, reps=1):
    import concourse.mybir as mybir
    import concourse.tile as tile
    from concourse import bacc
    from concourse.masks import make_identity

    dt = mybir.dt
    f32 = dt.float32
    f32r = dt.float32r
    AF = mybir.ActivationFunctionType
    X = mybir.AxisListType.X

    n_dev = 1 if local_sim else C
    nc = bacc.Bacc("TRN2", target_bir_lowering=False, debug=False, num_devices=n_dev)

    emb_i_sl = nc.dram_tensor("emb_i_sl", [SLICE, D], f32, kind="ExternalInput")
    emb_j_sl = nc.dram_tensor("emb_j_sl", [SLICE, D], f32, kind="ExternalInput")
    out = nc.dram_tensor("out", [1, 1], f32, kind="ExternalOutput")

    rg = [list(range(C))]

    with tile.TileContext(nc) as tc:
        with (
            tc.tile_pool(name="dram", bufs=1, space="DRAM") as dram,
            tc.tile_pool(name="persist", bufs=1) as persist,
            tc.tile_pool(name="scr", bufs=2) as scr,
            tc.tile_pool(name="exp", bufs=4) as expp,
            tc.tile_pool(name="ps_t", bufs=2, space="PSUM") as ps_t,
            tc.tile_pool(name="ps_g", bufs=2, space="PSUM") as ps_g,
            tc.tile_pool(name="ps_cs", bufs=1, space="PSUM") as ps_cs,
        ):
            ag_in = dram.tile([D, NJT], f32r, name="ag_in")
            ag_out = dram.tile([C * D, NJT], f32r, name="ag_out")
            ar_in = dram.tile([1, AR_LEN], f32, name="ar_in")
            ar_out = dram.tile([1, AR_LEN], f32, name="ar_out")

            def body():
                identity = persist.tile([P, P], f32, name="identity")
                make_identity(nc, identity[:])
                ones_f = persist.tile([P, 1], f32, name="ones_f")
                nc.gpsimd.memset(ones_f[:], 1.0)
                ones_r = persist.tile([P, 1], f32r, name="ones_r")
                nc.vector.tensor_copy(ones_r[:], ones_f[:])
                # dummy Ln pulls the natural_log_exp ACT table load off the
                # critical path (runs while the input DMAs are in flight)
                warm = persist.tile([P, 1], f32, name="warm")
                nc.scalar.activation(warm[:], ones_f[:], AF.Ln)

                # ---- j path first, so the AllGather launches early: load
                # emb_j slice, row norms, inv-norm via exp(-0.5*ln),
                # normalize, transpose to z_j^T, ship to the collective ----
                nat_j = persist.tile([P, MI, D], f32, name="nat_j")
                nc.sync.dma_start(
                    nat_j[:], emb_j_sl[:].rearrange("(q p) d -> p q d", p=P)
                )
                nsq_j = persist.tile([P, MI], f32, name="nsq_j")
                for q in range(MI):
                    sq = scr.tile([P, D], f32, name="sq")
                    nc.vector.tensor_mul(sq[:], nat_j[:, q, :], nat_j[:, q, :])
                    nc.vector.reduce_sum(nsq_j[:, q : q + 1], sq[:], axis=X)
                lnn_j = persist.tile([P, MI], f32, name="lnn_j")
                nc.scalar.activation(lnn_j[:], nsq_j[:], AF.Ln)
                invn_j = persist.tile([P, MI], f32, name="invn_j")
                nc.scalar.activation(invn_j[:], lnn_j[:], AF.Exp, scale=-0.5)

                zjT = [
                    persist.tile([P, SLICE], f32r, name=f"zjT{k}") for k in range(KC)
                ]
                for q in range(MI):
                    zj = scr.tile([P, D], f32, name="zj")
                    nc.vector.tensor_scalar_mul(
                        zj[:], nat_j[:, q, :], invn_j[:, q : q + 1]
                    )
                    for k in range(KC):
                        pst = ps_t.tile([P, P], f32, name="pst")
                        nc.tensor.transpose(
                            pst[:], zj[:, k * P : (k + 1) * P], identity[:]
                        )
                        nc.vector.tensor_copy(zjT[k][:, q * P : (q + 1) * P], pst[:])
                for k in range(KC):
                    nc.sync.dma_start(ag_in[k * P : (k + 1) * P, :], zjT[k][:])

                # ---- i path: load, norms, raw emb_i^T (inv_ni folded into
                # the exp scale later), pos diag dots ----
                nat_i = persist.tile([P, MI, D], f32, name="nat_i")
                nc.sync.dma_start(
                    nat_i[:], emb_i_sl[:].rearrange("(q p) d -> p q d", p=P)
                )
                nsq_i = persist.tile([P, MI], f32, name="nsq_i")
                for q in range(MI):
                    sq = scr.tile([P, D], f32, name="sq")
                    nc.vector.tensor_mul(sq[:], nat_i[:, q, :], nat_i[:, q, :])
                    nc.vector.reduce_sum(nsq_i[:, q : q + 1], sq[:], axis=X)
                lnn_i = persist.tile([P, MI], f32, name="lnn_i")
                nc.scalar.activation(lnn_i[:], nsq_i[:], AF.Ln)
                invn_i = persist.tile([P, MI], f32, name="invn_i")
                nc.scalar.activation(invn_i[:], lnn_i[:], AF.Exp, scale=-0.5)
                scale2 = persist.tile([P, MI], f32, name="scale2")
                nc.vector.tensor_scalar_mul(scale2[:], invn_i[:], 2.0)
                invij = persist.tile([P, MI], f32, name="invij")
                nc.vector.tensor_mul(invij[:], invn_i[:], invn_j[:])

                lhsT = [
                    persist.tile([P, SLICE], f32r, name=f"lhsT{k}") for k in range(KC)
                ]
                for q in range(MI):
                    for k in range(KC):
                        pst = ps_t.tile([P, P], f32, name="pst")
                        nc.tensor.transpose(
                            pst[:], nat_i[:, q, k * P : (k + 1) * P], identity[:]
                        )
                        nc.vector.tensor_copy(lhsT[k][:, q * P : (q + 1) * P], pst[:])

                rawdot = persist.tile([P, MI], f32, name="rawdot")
                for q in range(MI):
                    prod = scr.tile([P, D], f32, name="prod")
                    nc.vector.tensor_mul(prod[:], nat_i[:, q, :], nat_j[:, q, :])
                    nc.vector.reduce_sum(rawdot[:, q : q + 1], prod[:], axis=X)
                # pos4m4 = 4 * pos (pre-scaled for the row-term subtraction)
                pos4 = persist.tile([P, MI], f32, name="pos4")
                nc.vector.tensor_mul(pos4[:], rawdot[:], invij[:])
                pos4m4 = persist.tile([P, MI], f32, name="pos4m4")
                nc.vector.tensor_scalar_mul(pos4m4[:], pos4[:], 4.0)

                # ---- AllGather z_j^T -> full rhs [256, 4096] ----
                if local_sim:
                    for c2 in range(C):
                        nc.sync.dma_start(ag_out[c2 * D : (c2 + 1) * D, :], ag_in[:])
                else:
                    nc.gpsimd.collective_compute(
                        "AllGather",
                        mybir.AluOpType.bypass,
                        ins=[ag_in.opt()],
                        outs=[ag_out.opt()],
                        replica_groups=rg,
                    )
                rhs = [persist.tile([P, N], f32r, name=f"rhs{k}") for k in range(KC)]
                for c2 in range(C):
                    for k in range(KC):
                        nc.sync.dma_start(
                            rhs[k][:, c2 * NJT : (c2 + 1) * NJT],
                            ag_out[c2 * D + k * P : c2 * D + (k + 1) * P, :],
                        )

                # ---- main loop: S block, fused exp/rowsum, colsum MMs ----
                cs_sb = persist.tile([1, AR_LEN], f32, name="cs_sb")
                # only the pad past the P_c slot needs zeroing
                nc.gpsimd.memset(cs_sb[0:1, N + 1 : AR_LEN], 0.0)
                NJP = NJ // 2
                W = 2 * NJT
                rsparts = persist.tile([P, MI * NJP], f32, name="rsparts")
                for njp in range(NJP):
                    cs_h = [
                        ps_cs.tile([1, NJT], f32, name=f"cs_h{h}") for h in range(2)
                    ]
                    for mi in range(MI):
                        g = ps_g.tile([P, W], f32, name="g")
                        for h in range(2):
                            for k in range(KC):
                                nc.tensor.matmul(
                                    g[:, h * NJT : (h + 1) * NJT],
                                    lhsT[k][:, mi * P : (mi + 1) * P],
                                    rhs[k][
                                        :,
                                        (2 * njp + h) * NJT : (2 * njp + h + 1) * NJT,
                                    ],
                                    start=(k == 0),
                                    stop=(k == KC - 1),
                                )
                        e = expp.tile([P, W], f32r, name="e")
                        col = mi * NJP + njp
                        nc.scalar.activation(
                            e[:],
                            g[:],
                            AF.Exp,
                            scale=scale2[:, mi : mi + 1],
                            accum_out=rsparts[:, col : col + 1],
                        )
                        for h in range(2):
                            nc.tensor.matmul(
                                cs_h[h][:],
                                ones_r[:],
                                e[:, h * NJT : (h + 1) * NJT],
                                start=(mi == 0),
                                stop=(mi == MI - 1),
                                skip_group_check=True,
                            )
                    for h in range(2):
                        nc.vector.tensor_copy(
                            cs_sb[0:1, (2 * njp + h) * NJT : (2 * njp + h + 1) * NJT],
                            cs_h[h][:],
                        )

                # ---- per-core scalar P_c = sum(log(0.5*rowsum) - 4*pos) ----
                rs4 = persist.tile([P, MI], f32, name="rs4")
                nc.vector.reduce_sum(
                    rs4[:], rsparts[:].rearrange("p (m j) -> p m j", j=NJ // 2), axis=X
                )
                lg4 = persist.tile([P, MI], f32, name="lg4")
                nc.scalar.activation(lg4[:], rs4[:], AF.Ln, scale=0.5)
                rowterm = persist.tile([P, MI], f32, name="rowterm")
                nc.vector.tensor_sub(rowterm[:], lg4[:], pos4m4[:])
                rowv = persist.tile([P, 1], f32, name="rowv")
                nc.vector.reduce_sum(rowv[:], rowterm[:], axis=X)
                p_ps = ps_t.tile([1, 1], f32, name="p_ps", tag="pst")
                nc.tensor.matmul(p_ps[:], rowv[:], ones_f[:])
                nc.scalar.copy(cs_sb[0:1, N : N + 1], p_ps[:])

                # ---- AllReduce colsums + P_c ----
                nc.sync.dma_start(ar_in[:], cs_sb[:])
                if local_sim:
                    nc.sync.dma_start(ar_out[:], ar_in[:])
                else:
                    nc.gpsimd.collective_compute(
                        "AllReduce",
                        mybir.AluOpType.add,
                        ins=[ar_in.opt()],
                        outs=[ar_out.opt()],
                        replica_groups=rg,
                    )

                # ---- final loss (identical on every core) ----
                FW = N // P  # 32
                logs_in = persist.tile([P, FW], f32, name="logs_in")
                nc.sync.dma_start(
                    logs_in[:], ar_out[0:1, 0:N].rearrange("a (p f) -> (a p) f", p=P)
                )
                ptot = persist.tile([1, 1], f32, name="ptot")
                nc.sync.dma_start(ptot[:], ar_out[0:1, N : N + 1])
                lgc = persist.tile([P, FW], f32, name="lgc")
                nc.scalar.activation(lgc[:], logs_in[:], AF.Ln, scale=0.5)
                lgsum = persist.tile([P, 1], f32, name="lgsum")
                nc.vector.reduce_sum(lgsum[:], lgc[:], axis=X)
                l_ps = ps_t.tile([1, 1], f32, name="l_ps", tag="pst")
                nc.tensor.matmul(l_ps[:], lgsum[:], ones_f[:])
                lcol = persist.tile([1, 1], f32, name="lcol")
                nc.scalar.copy(lcol[:], l_ps[:])
                tot = persist.tile([1, 1], f32, name="tot")
                nc.vector.tensor_add(tot[:], ptot[:], lcol[:])
                loss = persist.tile([1, 1], f32, name="loss")
                nc.scalar.mul(loss[:], tot[:], 1.0 / (2.0 * N))
                nc.sync.dma_start(out[:], loss[:])

            for _rep in range(reps):
                body()

    nc.compile()
    return nc


def kernel(emb_i, emb_j):
    from concourse.bass_utils import run_bass_kernel_spmd

    if "nc" not in _CACHE:
        _CACHE["nc"] = _build_nc()
    nc = _CACHE["nc"]

    emb_i = np.ascontiguousarray(np.asarray(emb_i, dtype=np.float32))
    emb_j = np.ascontiguousarray(np.asarray(emb_j, dtype=np.float32))
    in_maps = [
        {
            "emb_i_sl": emb_i[c * SLICE : (c + 1) * SLICE],
            "emb_j_sl": emb_j[c * SLICE : (c + 1) * SLICE],
        }
        for c in range(C)
    ]
    res = run_bass_kernel_spmd(nc, in_maps, list(range(C)))
    val = np.asarray(res.results[0]["out"], dtype=np.float32)
    return val.reshape(())

